# revision 1
# baseline (speedup 1.0000x reference)
"""HBV hydrological model (nn_HBVMulTDET_WaterLoss) as a Bass/Tile kernel on
8 Trainium2 NeuronCores.

Strategy: pure data parallelism over the 4000 grid cells (500 cells/core).
Per-core layout: partition p in [0,125) holds 4 cells x 4 components = 16
state lanes in the free dim (flat index cl*4+m). The T=365 recurrence runs
as a fully unrolled instruction stream: the snow subsystem on GPSIMD (Pool),
the soil/response chains on DVE, ln/exp on the Scalar (ACT) engine, bulk
time-invariant precomputation (parameter scaling, rain/snow partitioning)
batched per time-chunk. Gamma unit-hydrograph weights are computed on host
(tiny [15,4000] preprocessing of conv_params); the routing convolution runs
on device.
"""
import math
import numpy as np

T_FULL = 365
NGRID = 4000
NCORES = 8
NSH = NGRID // NCORES      # 500 cells per core
PPART = 125                # partitions used
CL = 4                     # cells per partition
M = 4                      # nmul components
LENF = 15
NZ = 1e-5
TC = 32                    # time-chunk length

# (scale, bias) applied to raw params: p = raw*scale + bias.
# Index 10 (CFR) and 13 (C) are sign-folded (negated) for downstream fusions.
SCALE = {
    0: (5.0, 1.0),       # BETA
    1: (950.0, 50.0),    # FC
    2: (0.85, 0.05),     # K0
    3: (0.49, 0.01),     # K1
    4: (0.199, 0.001),   # K2
    5: (0.8, 0.2),       # LP
    6: (10.0, 0.0),      # PERC
    7: (100.0, 0.0),     # UZL
    8: (5.0, -2.5),      # TT
    9: (9.5, 0.5),       # CFMAX
    10: (-0.1, 0.0),     # CFRn = -CFR
    11: (0.2, 0.0),      # CWH
    12: (4.7, 0.3),      # BETAET
    13: (-1.0, 0.0),     # Cn = -C
}


def build_program(T=T_FULL, tc_len=TC):
    import concourse.bass as bass
    import concourse.bacc as bacc
    import concourse.mybir as mybir
    import concourse.tile as tile

    F32 = mybir.dt.float32
    op = mybir.AluOpType
    AF = mybir.ActivationFunctionType

    nc = bacc.Bacc("TRN2")
    pp = nc.declare_dram_parameter("pp", [14, PPART, T, CL, M], F32, isOutput=False)
    xf = nc.declare_dram_parameter("xf", [3, PPART, T, CL], F32, isOutput=False)
    uh = nc.declare_dram_parameter("uh", [PPART, LENF * CL], F32, isOutput=False)
    qr = nc.declare_dram_parameter("qr", [PPART, T, CL], F32, isOutput=True)

    chunks = [(t0, min(tc_len, T - t0)) for t0 in range(0, T, tc_len)]

    with tile.TileContext(nc) as tctx:
        with (
            tctx.tile_pool(name="par", bufs=2) as par_pool,
            tctx.tile_pool(name="blk", bufs=2) as blk_pool,
            tctx.tile_pool(name="st", bufs=4) as st_pool,
            tctx.tile_pool(name="per", bufs=1) as per_pool,
        ):
            V = nc.vector
            G = nc.gpsimd
            A = nc.scalar
            S = nc.sync

            def tt(eng, out, a, b, o):
                eng.tensor_tensor(out, a, b, o)

            Qfull = per_pool.tile([PPART, (LENF - 1 + T) * CL], F32)
            uh_t = per_pool.tile([PPART, LENF * CL], F32)
            S.dma_start(uh_t[:], uh[:])
            G.memset(Qfull[:, : (LENF - 1) * CL], 0.0)

            state = {}
            for s in ("SP", "MW", "SM", "SUZ", "SLZ"):
                t_ = st_pool.tile([PPART, 16], F32, tag=s)
                G.memset(t_[:], 0.001)
                state[s] = t_

            def nt(tag):
                return st_pool.tile([PPART, 16], F32, tag=tag, name=tag)

            def emit_response(p):
                """Response routine for step p['t'] (on DVE), emitted lazily
                inside step t+1's ACT wait windows."""
                if p is None:
                    return
                re_ = nt("re")
                tt(V, re_[:], p["rech"][:], p["exc"][:], op.add)
                SUZ1 = nt("SUZ1")
                tt(V, SUZ1[:], state["SUZ"][:], re_[:], op.add)
                PERCa = nt("PERCa")
                tt(V, PERCa[:], SUZ1[:], p["PERC"], op.min)
                SUZ2 = nt("SUZ2")
                tt(V, SUZ2[:], SUZ1[:], PERCa[:], op.subtract)
                qm = nt("qm")
                tt(V, qm[:], SUZ2[:], p["UZL"], op.max)
                q = nt("q")
                tt(V, q[:], qm[:], p["UZL"], op.subtract)
                Q0 = nt("Q0")
                tt(V, Q0[:], p["K0"], q[:], op.mult)
                SUZ3 = nt("SUZ3")
                tt(V, SUZ3[:], SUZ2[:], Q0[:], op.subtract)
                Q1 = nt("Q1")
                tt(V, Q1[:], p["K1"], SUZ3[:], op.mult)
                SUZn = nt("SUZ")
                tt(V, SUZn[:], SUZ3[:], Q1[:], op.subtract)
                state["SUZ"] = SUZn
                SLZ2 = nt("SLZ2")
                tt(V, SLZ2[:], p["SLZ1"][:], PERCa[:], op.add)
                Q2 = nt("Q2")
                tt(V, Q2[:], p["K2"], SLZ2[:], op.mult)
                SLZn = nt("SLZ")
                tt(V, SLZn[:], SLZ2[:], Q2[:], op.subtract)
                state["SLZ"] = SLZn
                Qa = nt("Qa")
                tt(V, Qa[:], Q0[:], Q1[:], op.add)
                Qb = nt("Qb")
                tt(V, Qb[:], Qa[:], Q2[:], op.add)
                t_ = p["t"]
                V.tensor_reduce(
                    Qfull[:, (LENF - 1 + t_) * CL : (LENF + t_) * CL],
                    Qb[:].rearrange("p (c m) -> p c m", m=M),
                    axis=mybir.AxisListType.X,
                    op=op.add,
                )

            pend = None

            for (t0, tcn) in chunks:
                n16 = tcn * 16
                # ---- chunk DMAs ----
                part = {}
                for k in range(14):
                    pt = par_pool.tile([PPART, tc_len * 16], F32, tag=f"par{k}",
                                       name=f"par{k}_{t0}")
                    S.dma_start(
                        pt[:, :n16].rearrange("p (t c m) -> p t c m", c=CL, m=M),
                        pp[k, :, t0 : t0 + tcn, :, :],
                    )
                    part[k] = pt
                xft = {}
                for c in range(3):
                    xt = blk_pool.tile([PPART, tc_len * CL], F32, tag=f"xf{c}",
                                       name=f"xf{c}_{t0}")
                    S.dma_start(
                        xt[:, : tcn * CL].rearrange("p (t c) -> p t c", c=CL),
                        xf[c, :, t0 : t0 + tcn, :],
                    )
                    xft[c] = xt

                # ---- parameter scaling in-place (ACT) ----
                for k, (sc_, bi_) in SCALE.items():
                    A.activation(part[k][:, :n16], part[k][:, :n16], AF.Copy,
                                 bias=float(bi_), scale=float(sc_))

                def bc4(xtile):
                    # [125, tcn*4] -> broadcast [125, tcn, 4, 4] over m
                    return (
                        xtile[:, : tcn * CL]
                        .rearrange("p (t c) -> p t c", c=CL)
                        .unsqueeze(3)
                        .to_broadcast((PPART, tcn, CL, M))
                    )

                def f4(btile):
                    return btile[:, :n16].rearrange(
                        "p (t c m) -> p t c m", c=CL, m=M
                    )

                Pb = bc4(xft[0])
                TAb = bc4(xft[1])
                PETb = bc4(xft[2])

                def bt(tag):
                    return blk_pool.tile([PPART, tc_len * 16], F32, tag=tag, name=tag)

                # ---- bulk derived (Pool) ----
                Gt = bt("G")
                tt(G, f4(Gt), TAb, f4(part[8]), op.subtract)       # Ta - TT
                maskt = bt("mask")
                tt(V, f4(maskt), TAb, f4(part[8]), op.is_ge)       # DVE: Pool lacks is_ge
                RAIN = bt("RAIN")
                tt(G, f4(RAIN), f4(maskt), Pb, op.mult)
                SNOW = bt("SNOW")
                tt(G, f4(SNOW), Pb, f4(RAIN), op.subtract)
                Gc = bt("Gc")
                tt(G, Gc[:, :n16], part[9][:, :n16], Gt[:, :n16], op.mult)
                G.tensor_scalar_max(Gc[:, :n16], Gc[:, :n16], 0.0)
                CFMXn = bt("CFMXn")
                tt(G, CFMXn[:, :n16], part[10][:, :n16], part[9][:, :n16], op.mult)
                Rc = bt("Rc")
                tt(G, Rc[:, :n16], CFMXn[:, :n16], Gt[:, :n16], op.mult)
                G.tensor_scalar_max(Rc[:, :n16], Rc[:, :n16], 0.0)
                # ---- bulk derived (DVE) ----
                FCinv = bt("FCinv")
                V.reciprocal(FCinv[:, :n16], part[1][:, :n16])
                LPFC = bt("LPFC")
                tt(V, LPFC[:, :n16], part[5][:, :n16], part[1][:, :n16], op.mult)
                LPFCinv = bt("LPFCinv")
                V.reciprocal(LPFCinv[:, :n16], LPFC[:, :n16])

                # ---- sequential steps ----
                for ti in range(tcn):
                    t = t0 + ti
                    sl = slice(ti * 16, (ti + 1) * 16)

                    def ps(k):
                        return part[k][:, sl]

                    # -- snow subsystem (Pool; no tensor-tensor min on Pool,
                    #    so min(a,b) = a - relu(a-b)) --
                    SP1 = nt("SP1")
                    tt(G, SP1[:], state["SP"][:], SNOW[:, sl], op.add)
                    md = nt("md")
                    tt(G, md[:], Gc[:, sl], SP1[:], op.subtract)
                    G.tensor_scalar_max(md[:], md[:], 0.0)
                    melt = nt("melt")
                    tt(G, melt[:], Gc[:, sl], md[:], op.subtract)
                    MW1 = nt("MW1")
                    tt(G, MW1[:], state["MW"][:], melt[:], op.add)
                    SP2 = nt("SP2")
                    tt(G, SP2[:], SP1[:], melt[:], op.subtract)
                    G.tensor_scalar_max(SP2[:], SP2[:], NZ)
                    rd = nt("rd")
                    tt(G, rd[:], Rc[:, sl], MW1[:], op.subtract)
                    G.tensor_scalar_max(rd[:], rd[:], 0.0)
                    rfz = nt("rfz")
                    tt(G, rfz[:], Rc[:, sl], rd[:], op.subtract)
                    SP3 = nt("SP")
                    tt(G, SP3[:], SP2[:], rfz[:], op.add)
                    state["SP"] = SP3
                    MW2 = nt("MW2")
                    tt(G, MW2[:], MW1[:], rfz[:], op.subtract)
                    G.tensor_scalar_max(MW2[:], MW2[:], NZ)
                    W = nt("W")
                    tt(G, W[:], ps(11), SP3[:], op.mult)
                    tos = nt("tos")
                    tt(G, tos[:], MW2[:], W[:], op.subtract)
                    G.tensor_scalar_max(tos[:], tos[:], 0.0)
                    MW3 = nt("MW")
                    tt(G, MW3[:], MW2[:], tos[:], op.subtract)
                    G.tensor_scalar_max(MW3[:], MW3[:], NZ)
                    state["MW"] = MW3
                    wi = nt("wi")
                    tt(G, wi[:], RAIN[:, sl], tos[:], op.add)

                    # -- soil chain (DVE + ACT) --
                    SM = state["SM"]
                    r = nt("r")
                    tt(V, r[:], SM[:], FCinv[:, sl], op.mult)
                    lr = nt("lr")
                    A.activation(lr[:], r[:], AF.Ln)
                    # fill the ACT window with the previous step's response
                    emit_response(pend)
                    e = nt("e")
                    tt(V, e[:], ps(0), lr[:], op.mult)
                    x1 = nt("x1")
                    A.activation(x1[:], e[:], AF.Exp)
                    SMa = nt("SMa")
                    tt(V, SMa[:], SM[:], wi[:], op.add)
                    rech = nt("rech")
                    V.scalar_tensor_tensor(rech[:], x1[:], 1.0, wi[:], op.min, op.mult)
                    SM1 = nt("SM1")
                    tt(V, SM1[:], SMa[:], rech[:], op.subtract)
                    SMc = nt("SMc")
                    tt(V, SMc[:], SM1[:], ps(1), op.min)
                    exc = nt("exc")
                    tt(V, exc[:], SM1[:], SMc[:], op.subtract)
                    V.tensor_scalar_max(SMc[:], SMc[:], NZ)
                    r2 = nt("r2")
                    tt(V, r2[:], SMc[:], LPFCinv[:, sl], op.mult)
                    l2 = nt("l2")
                    A.activation(l2[:], r2[:], AF.Ln)
                    e2 = nt("e2")
                    tt(V, e2[:], ps(12), l2[:], op.mult)
                    x2 = nt("x2")
                    A.activation(x2[:], e2[:], AF.Exp)
                    pe = nt("pe")
                    V.scalar_tensor_tensor(
                        pe[:].rearrange("p (c m) -> p c m", m=M),
                        x2[:].rearrange("p (c m) -> p c m", m=M), 1.0,
                        PETb[:, ti, :, :],
                        op.min, op.mult,
                    )
                    ET = nt("ET")
                    tt(V, ET[:], SMc[:], pe[:], op.min)
                    SM3 = nt("SM3")
                    tt(V, SM3[:], SMc[:], ET[:], op.subtract)
                    V.tensor_scalar_max(SM3[:], SM3[:], NZ)
                    r3 = nt("r3")
                    tt(V, r3[:], SM3[:], FCinv[:, sl], op.mult)
                    V.tensor_scalar(r3[:], r3[:], 1.0, 1.0, op.min, op.subtract)
                    co = nt("co")
                    tt(V, co[:], ps(13), r3[:], op.mult)
                    cap = nt("cap")
                    V.scalar_tensor_tensor(cap[:], co[:], 1.0, state["SLZ"][:],
                                           op.min, op.mult)
                    SM4 = nt("SM")
                    tt(V, SM4[:], SM3[:], cap[:], op.add)
                    state["SM"] = SM4
                    SLZ1 = nt("SLZ1")
                    tt(V, SLZ1[:], state["SLZ"][:], cap[:], op.subtract)
                    V.tensor_scalar_max(SLZ1[:], SLZ1[:], NZ)

                    pend = {
                        "t": t, "rech": rech, "exc": exc, "SLZ1": SLZ1,
                        "PERC": ps(6), "UZL": ps(7), "K0": ps(2),
                        "K1": ps(3), "K2": ps(4),
                    }

            emit_response(pend)

            # ---- gamma-UH routing (DVE, bulk) ----
            Qr = per_pool.tile([PPART, T * CL], F32)
            prod = per_pool.tile([PPART, T * CL], F32)

            def qr4(ap_):
                return ap_.rearrange("p (t c) -> p t c", c=CL)

            for k in range(LENF):
                sh = Qfull[:, (LENF - 1 - k) * CL : (LENF - 1 - k + T) * CL]
                uhk = (
                    uh_t[:, k * CL : (k + 1) * CL]
                    .unsqueeze(1)
                    .to_broadcast((PPART, T, CL))
                )
                if k == 0:
                    tt(V, qr4(Qr[:]), uhk, qr4(sh), op.mult)
                else:
                    tt(V, qr4(prod[:]), uhk, qr4(sh), op.mult)
                    tt(V, qr4(Qr[:]), qr4(Qr[:]), qr4(prod[:]), op.add)

            S.dma_start(qr[:, :, :], Qr[:].rearrange("p (t c) -> p t c", c=CL))

    return nc


# ---------------- host-side packing ----------------

def pack_inputs(x_hydro_model, params_raw, conv_params_hydro):
    T = x_hydro_model.shape[0]
    f32 = np.float32
    x = np.ascontiguousarray(x_hydro_model, dtype=f32)
    xs = x.reshape(T, NCORES, PPART, CL, 3).transpose(1, 4, 2, 0, 3)
    pr = np.ascontiguousarray(params_raw[:, :, :14, :], dtype=f32)
    prs = pr.reshape(T, NCORES, PPART, CL, 14, M).transpose(1, 4, 2, 0, 3, 5)

    conv = np.asarray(conv_params_hydro, dtype=np.float64)
    a = conv[:, 0] * 2.9
    b = conv[:, 1] * 6.5
    aa = np.maximum(a, 0) + 0.1
    theta = np.maximum(b, 0) + 0.5
    tgrid = np.arange(0.5, float(LENF), dtype=np.float64)[:, None]
    lg = np.array([math.lgamma(v) for v in aa])
    w = np.exp(-lg) / theta ** aa * tgrid ** (aa - 1.0) * np.exp(-tgrid / theta)
    w = w / w.sum(0)
    UH = (w * (1.0 / M)).astype(f32)  # [LENF, NGRID], mean-over-M folded in
    uh_c = UH.reshape(LENF, NCORES, PPART, CL).transpose(1, 2, 0, 3)

    in_maps = []
    for i in range(NCORES):
        in_maps.append({
            "pp": np.ascontiguousarray(prs[i]),
            "xf": np.ascontiguousarray(xs[i]),
            "uh": np.ascontiguousarray(uh_c[i]).reshape(PPART, LENF * CL),
        })
    return in_maps


def unpack_outputs(results, T):
    out = np.empty((T, NGRID), np.float32)
    for i in range(NCORES):
        q = results[i]["qr"].reshape(PPART, T, CL)
        out[:, i * NSH : (i + 1) * NSH] = q.transpose(1, 0, 2).reshape(T, NSH)
    return out


_PROG_CACHE = {}


def kernel(x_hydro_model, params_raw, conv_params_hydro):
    from concourse.bass_utils import run_bass_kernel_spmd

    T = x_hydro_model.shape[0]
    key = T
    if key not in _PROG_CACHE:
        _PROG_CACHE[key] = build_program(T=T)
    nc = _PROG_CACHE[key]
    if not nc.is_finalized():
        nc.finalize()
    in_maps = pack_inputs(x_hydro_model, params_raw, conv_params_hydro)
    res = run_bass_kernel_spmd(nc, in_maps, list(range(NCORES)))
    return unpack_outputs(res.results, T)



# revision 2
# speedup vs baseline: 1.5399x; 1.5399x over previous
"""HBV hydrological model (nn_HBVMulTDET_WaterLoss) as a Bass/Tile kernel on
8 Trainium2 NeuronCores.

Strategy: pure data parallelism over the 4000 grid cells (500 cells/core).
Per-core layout: partition p in [0,125) holds 4 cells x 4 components = 16
state lanes in the free dim (flat index cl*4+m). The T=365 recurrence runs
as a fully unrolled instruction stream: the snow subsystem on GPSIMD (Pool),
the soil/response chains on DVE, ln/exp on the Scalar (ACT) engine, bulk
time-invariant precomputation (parameter scaling, rain/snow partitioning)
batched per time-chunk. Gamma unit-hydrograph weights are computed on host
(tiny [15,4000] preprocessing of conv_params); the routing convolution runs
on device.
"""
import math
import numpy as np

T_FULL = 365
NGRID = 4000
NCORES = 8
NSH = NGRID // NCORES      # 500 cells per core
PPART = 125                # partitions used
CL = 4                     # cells per partition
M = 4                      # nmul components
LENF = 15
NZ = 1e-5
TC = 32                    # time-chunk length

# (scale, bias) applied to raw params: p = raw*scale + bias.
# Index 10 (CFR) and 13 (C) are sign-folded (negated) for downstream fusions.
SCALE = {
    0: (5.0, 1.0),       # BETA
    1: (950.0, 50.0),    # FC
    2: (0.85, 0.05),     # K0
    3: (0.49, 0.01),     # K1
    4: (0.199, 0.001),   # K2
    5: (0.8, 0.2),       # LP
    6: (10.0, 0.0),      # PERC
    7: (100.0, 0.0),     # UZL
    8: (5.0, -2.5),      # TT
    9: (9.5, 0.5),       # CFMAX
    10: (-0.1, 0.0),     # CFRn = -CFR
    11: (0.2, 0.0),      # CWH
    12: (4.7, 0.3),      # BETAET
    13: (-1.0, 0.0),     # Cn = -C
}


def build_program(T=T_FULL, tc_len=TC):
    import concourse.bass as bass
    import concourse.bacc as bacc
    import concourse.mybir as mybir
    import concourse.tile as tile

    F32 = mybir.dt.float32
    op = mybir.AluOpType
    AF = mybir.ActivationFunctionType

    nc = bacc.Bacc("TRN2")
    pp = nc.declare_dram_parameter("pp", [14, PPART, T, CL, M], F32, isOutput=False)
    xf = nc.declare_dram_parameter("xf", [3, PPART, T, CL], F32, isOutput=False)
    uh = nc.declare_dram_parameter("uh", [PPART, LENF * CL], F32, isOutput=False)
    qr = nc.declare_dram_parameter("qr", [PPART, T, CL], F32, isOutput=True)

    chunks = [(t0, min(tc_len, T - t0)) for t0 in range(0, T, tc_len)]

    with tile.TileContext(nc) as tctx:
        with (
            tctx.tile_pool(name="par", bufs=2) as par_pool,
            tctx.tile_pool(name="blk", bufs=2) as blk_pool,
            tctx.tile_pool(name="st", bufs=4) as st_pool,
            tctx.tile_pool(name="per", bufs=1) as per_pool,
        ):
            # Seed the ACT table that contains BOTH Ln and Exp
            # (natural_log_exp_and_others, set id 6). Without this the
            # table-load pass ping-pongs between the exp-only (0) and
            # ln-only (5) tables: 1461 ACT_TABLE_LOADs x 1.28us.
            nc.scalar.add_instruction(
                mybir.InstLoadActFuncSet(
                    name=nc.get_next_instruction_name(),
                    act_func_set_id=6, ins=[], outs=[],
                )
            )
            V = nc.vector
            G = nc.gpsimd
            A = nc.scalar
            S = nc.sync

            def tt(eng, out, a, b, o):
                eng.tensor_tensor(out, a, b, o)

            Qfull = per_pool.tile([PPART, (LENF - 1 + T) * CL], F32)
            uh_t = per_pool.tile([PPART, LENF * CL], F32)
            S.dma_start(uh_t[:], uh[:])
            G.memset(Qfull[:, : (LENF - 1) * CL], 0.0)

            state = {}
            for s in ("SP", "MW", "SM", "SUZ", "SLZ"):
                t_ = st_pool.tile([PPART, 16], F32, tag=s)
                G.memset(t_[:], 0.001)
                state[s] = t_

            def nt(tag):
                return st_pool.tile([PPART, 16], F32, tag=tag, name=tag)

            def emit_response(p):
                """Response routine for step p['t'] (on DVE), emitted lazily
                inside step t+1's ACT wait windows."""
                if p is None:
                    return
                re_ = nt("re")
                tt(V, re_[:], p["rech"][:], p["exc"][:], op.add)
                SUZ1 = nt("SUZ1")
                tt(V, SUZ1[:], state["SUZ"][:], re_[:], op.add)
                PERCa = nt("PERCa")
                tt(V, PERCa[:], SUZ1[:], p["PERC"], op.min)
                SUZ2 = nt("SUZ2")
                tt(V, SUZ2[:], SUZ1[:], PERCa[:], op.subtract)
                qm = nt("qm")
                tt(V, qm[:], SUZ2[:], p["UZL"], op.max)
                q = nt("q")
                tt(V, q[:], qm[:], p["UZL"], op.subtract)
                Q0 = nt("Q0")
                tt(V, Q0[:], p["K0"], q[:], op.mult)
                SUZ3 = nt("SUZ3")
                tt(V, SUZ3[:], SUZ2[:], Q0[:], op.subtract)
                Q1 = nt("Q1")
                tt(V, Q1[:], p["K1"], SUZ3[:], op.mult)
                SUZn = nt("SUZ")
                tt(V, SUZn[:], SUZ3[:], Q1[:], op.subtract)
                state["SUZ"] = SUZn
                SLZ2 = nt("SLZ2")
                tt(V, SLZ2[:], p["SLZ1"][:], PERCa[:], op.add)
                Q2 = nt("Q2")
                tt(V, Q2[:], p["K2"], SLZ2[:], op.mult)
                SLZn = nt("SLZ")
                tt(V, SLZn[:], SLZ2[:], Q2[:], op.subtract)
                state["SLZ"] = SLZn
                Qa = nt("Qa")
                tt(V, Qa[:], Q0[:], Q1[:], op.add)
                Qb = nt("Qb")
                tt(V, Qb[:], Qa[:], Q2[:], op.add)
                t_ = p["t"]
                V.tensor_reduce(
                    Qfull[:, (LENF - 1 + t_) * CL : (LENF + t_) * CL],
                    Qb[:].rearrange("p (c m) -> p c m", m=M),
                    axis=mybir.AxisListType.X,
                    op=op.add,
                )

            pend = None

            for (t0, tcn) in chunks:
                n16 = tcn * 16
                # ---- chunk DMAs ----
                part = {}
                for k in range(14):
                    pt = par_pool.tile([PPART, tc_len * 16], F32, tag=f"par{k}",
                                       name=f"par{k}_{t0}")
                    S.dma_start(
                        pt[:, :n16].rearrange("p (t c m) -> p t c m", c=CL, m=M),
                        pp[k, :, t0 : t0 + tcn, :, :],
                    )
                    part[k] = pt
                xft = {}
                for c in range(3):
                    xt = blk_pool.tile([PPART, tc_len * CL], F32, tag=f"xf{c}",
                                       name=f"xf{c}_{t0}")
                    S.dma_start(
                        xt[:, : tcn * CL].rearrange("p (t c) -> p t c", c=CL),
                        xf[c, :, t0 : t0 + tcn, :],
                    )
                    xft[c] = xt

                # ---- parameter scaling in-place (ACT) ----
                for k, (sc_, bi_) in SCALE.items():
                    A.activation(part[k][:, :n16], part[k][:, :n16], AF.Copy,
                                 bias=float(bi_), scale=float(sc_))

                def bc4(xtile):
                    # [125, tcn*4] -> broadcast [125, tcn, 4, 4] over m
                    return (
                        xtile[:, : tcn * CL]
                        .rearrange("p (t c) -> p t c", c=CL)
                        .unsqueeze(3)
                        .to_broadcast((PPART, tcn, CL, M))
                    )

                def f4(btile):
                    return btile[:, :n16].rearrange(
                        "p (t c m) -> p t c m", c=CL, m=M
                    )

                Pb = bc4(xft[0])
                TAb = bc4(xft[1])
                PETb = bc4(xft[2])

                def bt(tag):
                    return blk_pool.tile([PPART, tc_len * 16], F32, tag=tag, name=tag)

                # ---- bulk derived (Pool) ----
                Gt = bt("G")
                tt(G, f4(Gt), TAb, f4(part[8]), op.subtract)       # Ta - TT
                maskt = bt("mask")
                tt(V, f4(maskt), TAb, f4(part[8]), op.is_ge)       # DVE: Pool lacks is_ge
                RAIN = bt("RAIN")
                tt(G, f4(RAIN), f4(maskt), Pb, op.mult)
                SNOW = bt("SNOW")
                tt(G, f4(SNOW), Pb, f4(RAIN), op.subtract)
                Gc = bt("Gc")
                tt(G, Gc[:, :n16], part[9][:, :n16], Gt[:, :n16], op.mult)
                G.tensor_scalar_max(Gc[:, :n16], Gc[:, :n16], 0.0)
                CFMXn = bt("CFMXn")
                tt(G, CFMXn[:, :n16], part[10][:, :n16], part[9][:, :n16], op.mult)
                Rc = bt("Rc")
                tt(G, Rc[:, :n16], CFMXn[:, :n16], Gt[:, :n16], op.mult)
                G.tensor_scalar_max(Rc[:, :n16], Rc[:, :n16], 0.0)
                # ---- bulk derived (DVE) ----
                FCinv = bt("FCinv")
                V.reciprocal(FCinv[:, :n16], part[1][:, :n16])
                LPFC = bt("LPFC")
                tt(V, LPFC[:, :n16], part[5][:, :n16], part[1][:, :n16], op.mult)
                LPFCinv = bt("LPFCinv")
                V.reciprocal(LPFCinv[:, :n16], LPFC[:, :n16])

                # ---- sequential steps ----
                for ti in range(tcn):
                    t = t0 + ti
                    sl = slice(ti * 16, (ti + 1) * 16)

                    def ps(k):
                        return part[k][:, sl]

                    # -- snow subsystem (Pool; no tensor-tensor min on Pool,
                    #    so min(a,b) = a - relu(a-b)) --
                    SP1 = nt("SP1")
                    tt(G, SP1[:], state["SP"][:], SNOW[:, sl], op.add)
                    md = nt("md")
                    tt(G, md[:], Gc[:, sl], SP1[:], op.subtract)
                    G.tensor_scalar_max(md[:], md[:], 0.0)
                    melt = nt("melt")
                    tt(G, melt[:], Gc[:, sl], md[:], op.subtract)
                    MW1 = nt("MW1")
                    tt(G, MW1[:], state["MW"][:], melt[:], op.add)
                    SP2 = nt("SP2")
                    tt(G, SP2[:], SP1[:], melt[:], op.subtract)
                    G.tensor_scalar_max(SP2[:], SP2[:], NZ)
                    rd = nt("rd")
                    tt(G, rd[:], Rc[:, sl], MW1[:], op.subtract)
                    G.tensor_scalar_max(rd[:], rd[:], 0.0)
                    rfz = nt("rfz")
                    tt(G, rfz[:], Rc[:, sl], rd[:], op.subtract)
                    SP3 = nt("SP")
                    tt(G, SP3[:], SP2[:], rfz[:], op.add)
                    state["SP"] = SP3
                    MW2 = nt("MW2")
                    tt(G, MW2[:], MW1[:], rfz[:], op.subtract)
                    G.tensor_scalar_max(MW2[:], MW2[:], NZ)
                    W = nt("W")
                    tt(G, W[:], ps(11), SP3[:], op.mult)
                    tos = nt("tos")
                    tt(G, tos[:], MW2[:], W[:], op.subtract)
                    G.tensor_scalar_max(tos[:], tos[:], 0.0)
                    MW3 = nt("MW")
                    tt(G, MW3[:], MW2[:], tos[:], op.subtract)
                    G.tensor_scalar_max(MW3[:], MW3[:], NZ)
                    state["MW"] = MW3
                    wi = nt("wi")
                    tt(G, wi[:], RAIN[:, sl], tos[:], op.add)

                    # -- soil chain (DVE + ACT) --
                    SM = state["SM"]
                    r = nt("r")
                    tt(V, r[:], SM[:], FCinv[:, sl], op.mult)
                    lr = nt("lr")
                    A.activation(lr[:], r[:], AF.Ln)
                    # fill the ACT window with the previous step's response
                    emit_response(pend)
                    e = nt("e")
                    tt(V, e[:], ps(0), lr[:], op.mult)
                    x1 = nt("x1")
                    A.activation(x1[:], e[:], AF.Exp)
                    SMa = nt("SMa")
                    tt(V, SMa[:], SM[:], wi[:], op.add)
                    rech = nt("rech")
                    V.scalar_tensor_tensor(rech[:], x1[:], 1.0, wi[:], op.min, op.mult)
                    SM1 = nt("SM1")
                    tt(V, SM1[:], SMa[:], rech[:], op.subtract)
                    SMc = nt("SMc")
                    tt(V, SMc[:], SM1[:], ps(1), op.min)
                    exc = nt("exc")
                    tt(V, exc[:], SM1[:], SMc[:], op.subtract)
                    V.tensor_scalar_max(SMc[:], SMc[:], NZ)
                    r2 = nt("r2")
                    tt(V, r2[:], SMc[:], LPFCinv[:, sl], op.mult)
                    l2 = nt("l2")
                    A.activation(l2[:], r2[:], AF.Ln)
                    e2 = nt("e2")
                    tt(V, e2[:], ps(12), l2[:], op.mult)
                    x2 = nt("x2")
                    A.activation(x2[:], e2[:], AF.Exp)
                    pe = nt("pe")
                    V.scalar_tensor_tensor(
                        pe[:].rearrange("p (c m) -> p c m", m=M),
                        x2[:].rearrange("p (c m) -> p c m", m=M), 1.0,
                        PETb[:, ti, :, :],
                        op.min, op.mult,
                    )
                    ET = nt("ET")
                    tt(V, ET[:], SMc[:], pe[:], op.min)
                    SM3 = nt("SM3")
                    tt(V, SM3[:], SMc[:], ET[:], op.subtract)
                    V.tensor_scalar_max(SM3[:], SM3[:], NZ)
                    r3 = nt("r3")
                    tt(V, r3[:], SM3[:], FCinv[:, sl], op.mult)
                    V.tensor_scalar(r3[:], r3[:], 1.0, 1.0, op.min, op.subtract)
                    co = nt("co")
                    tt(V, co[:], ps(13), r3[:], op.mult)
                    cap = nt("cap")
                    V.scalar_tensor_tensor(cap[:], co[:], 1.0, state["SLZ"][:],
                                           op.min, op.mult)
                    SM4 = nt("SM")
                    tt(V, SM4[:], SM3[:], cap[:], op.add)
                    state["SM"] = SM4
                    SLZ1 = nt("SLZ1")
                    tt(V, SLZ1[:], state["SLZ"][:], cap[:], op.subtract)
                    V.tensor_scalar_max(SLZ1[:], SLZ1[:], NZ)

                    pend = {
                        "t": t, "rech": rech, "exc": exc, "SLZ1": SLZ1,
                        "PERC": ps(6), "UZL": ps(7), "K0": ps(2),
                        "K1": ps(3), "K2": ps(4),
                    }

            emit_response(pend)

            # ---- gamma-UH routing (DVE, bulk) ----
            Qr = per_pool.tile([PPART, T * CL], F32)
            prod = per_pool.tile([PPART, T * CL], F32)

            def qr4(ap_):
                return ap_.rearrange("p (t c) -> p t c", c=CL)

            for k in range(LENF):
                sh = Qfull[:, (LENF - 1 - k) * CL : (LENF - 1 - k + T) * CL]
                uhk = (
                    uh_t[:, k * CL : (k + 1) * CL]
                    .unsqueeze(1)
                    .to_broadcast((PPART, T, CL))
                )
                if k == 0:
                    tt(V, qr4(Qr[:]), uhk, qr4(sh), op.mult)
                else:
                    tt(V, qr4(prod[:]), uhk, qr4(sh), op.mult)
                    tt(V, qr4(Qr[:]), qr4(Qr[:]), qr4(prod[:]), op.add)

            S.dma_start(qr[:, :, :], Qr[:].rearrange("p (t c) -> p t c", c=CL))

    return nc


# ---------------- host-side packing ----------------

def pack_inputs(x_hydro_model, params_raw, conv_params_hydro):
    T = x_hydro_model.shape[0]
    f32 = np.float32
    x = np.ascontiguousarray(x_hydro_model, dtype=f32)
    xs = x.reshape(T, NCORES, PPART, CL, 3).transpose(1, 4, 2, 0, 3)
    pr = np.ascontiguousarray(params_raw[:, :, :14, :], dtype=f32)
    prs = pr.reshape(T, NCORES, PPART, CL, 14, M).transpose(1, 4, 2, 0, 3, 5)

    conv = np.asarray(conv_params_hydro, dtype=np.float64)
    a = conv[:, 0] * 2.9
    b = conv[:, 1] * 6.5
    aa = np.maximum(a, 0) + 0.1
    theta = np.maximum(b, 0) + 0.5
    tgrid = np.arange(0.5, float(LENF), dtype=np.float64)[:, None]
    lg = np.array([math.lgamma(v) for v in aa])
    w = np.exp(-lg) / theta ** aa * tgrid ** (aa - 1.0) * np.exp(-tgrid / theta)
    w = w / w.sum(0)
    UH = (w * (1.0 / M)).astype(f32)  # [LENF, NGRID], mean-over-M folded in
    uh_c = UH.reshape(LENF, NCORES, PPART, CL).transpose(1, 2, 0, 3)

    in_maps = []
    for i in range(NCORES):
        in_maps.append({
            "pp": np.ascontiguousarray(prs[i]),
            "xf": np.ascontiguousarray(xs[i]),
            "uh": np.ascontiguousarray(uh_c[i]).reshape(PPART, LENF * CL),
        })
    return in_maps


def unpack_outputs(results, T):
    out = np.empty((T, NGRID), np.float32)
    for i in range(NCORES):
        q = results[i]["qr"].reshape(PPART, T, CL)
        out[:, i * NSH : (i + 1) * NSH] = q.transpose(1, 0, 2).reshape(T, NSH)
    return out


_PROG_CACHE = {}


def kernel(x_hydro_model, params_raw, conv_params_hydro):
    from concourse.bass_utils import run_bass_kernel_spmd

    T = x_hydro_model.shape[0]
    key = T
    if key not in _PROG_CACHE:
        _PROG_CACHE[key] = build_program(T=T)
    nc = _PROG_CACHE[key]
    if not nc.is_finalized():
        nc.finalize()
    in_maps = pack_inputs(x_hydro_model, params_raw, conv_params_hydro)
    res = run_bass_kernel_spmd(nc, in_maps, list(range(NCORES)))
    return unpack_outputs(res.results, T)



# revision 6
# speedup vs baseline: 1.6364x; 1.0627x over previous
"""HBV hydrological model (nn_HBVMulTDET_WaterLoss) as a Bass/Tile kernel on
8 Trainium2 NeuronCores.

Strategy: pure data parallelism over the 4000 grid cells (500 cells/core).
Per-core layout: partition p in [0,125) holds 4 cells x 4 components = 16
state lanes in the free dim. The T=365 recurrence is a fully unrolled
instruction stream balanced across three engines:
  - DVE: the soil-moisture critical cycle (pow via ln/exp affine, recharge,
    evap, capillary) with NZ-clamps fused into scalar_tensor_tensor ops.
  - Pool (GPSIMD): snow pack/meltwater scan + upper/lower-zone response,
    using only tensor_tensor add/sub/mult (the only TT ops Pool supports).
  - ACT: all max(x,0)-style clamps as Relu, plus Ln/Exp for the two powers.
    A single activation table (natural_log_exp_and_others, set id 6) is
    seeded explicitly so the compiler never reloads tables (1.28us each).
Algebraic restructurings vs the straight reference:
  - recharge/excess are never materialized: SUZ1 = (SUZ + SM + wi) - SMc.
  - capillary's min(.,1) and relu are provably no-ops (C<=1, SM3<=FC);
    SM4 and SLZ1 are computed from SLZ-linearized forms
    SM4 = max(SM3',NZ)*(1 - C*SLZ/FC) + C*SLZ, SLZ1 = SLZ*(1-C) + s*C*SLZ/FC
    whose SLZ-dependent factors are computed off the critical path.
  - evap factor uses SM1 instead of min(SM1,FC): identical after clip-to-1.
  - snow states are stored shifted by -NEARZERO so every clamp is a Relu.
  - x^b = exp(b*ln(x) - b*ln(FC)) with b*ln(FC) hoisted to bulk precompute.
Gamma unit-hydrograph weights are computed on host (tiny [15,4000]
preprocessing of conv_params); the routing convolution runs on device.
"""
import math
import numpy as np

T_FULL = 365
NGRID = 4000
NCORES = 8
NSH = NGRID // NCORES      # 500 cells per core
PPART = 125                # partitions used
CL = 4                     # cells per partition
M = 4                      # nmul components
LENF = 15
NZ = 1e-5
TC = 32                    # time-chunk length

# pp rows (param index in params_raw, scale, bias); K1/K2 ship separately
# pre-scaled as the packed "kk" tensor. CFR is sign-folded (negated).
PP_ROWS = [
    (0, 5.0, 1.0),       # 0 BETA
    (1, 950.0, 50.0),    # 1 FC
    (2, 0.85, 0.05),     # 2 K0
    (5, 0.8, 0.2),       # 3 LP
    (6, 10.0, 0.0),      # 4 PERC
    (7, 100.0, 0.0),     # 5 UZL
    (8, 5.0, -2.5),      # 6 TT
    (9, 9.5, 0.5),       # 7 CFMAX
    (10, -0.1, 0.0),     # 8 CFRn = -CFR
    (11, 0.2, 0.0),      # 9 CWH
    (12, 4.7, 0.3),      # 10 BETAET
    (13, 1.0, 0.0),      # 11 C
]
I_BETA, I_FC, I_K0, I_LP, I_PERC, I_UZL, I_TT, I_CFMAX, I_CFRN, I_CWH, \
    I_BETAET, I_C = range(12)


def build_program(T=T_FULL, tc_len=TC):
    import concourse.bass as bass
    import concourse.bacc as bacc
    import concourse.mybir as mybir
    import concourse.tile as tile

    F32 = mybir.dt.float32
    op = mybir.AluOpType
    AF = mybir.ActivationFunctionType

    nc = bacc.Bacc("TRN2")
    pp = nc.declare_dram_parameter("pp", [12, PPART, T, CL, M], F32, isOutput=False)
    kk = nc.declare_dram_parameter("kk", [PPART, T, 2, CL, M], F32, isOutput=False)
    xf = nc.declare_dram_parameter("xf", [3, PPART, T, CL], F32, isOutput=False)
    uh = nc.declare_dram_parameter("uh", [PPART, LENF * CL], F32, isOutput=False)
    qr = nc.declare_dram_parameter("qr", [PPART, T, CL], F32, isOutput=True)

    chunks = [(t0, min(tc_len, T - t0)) for t0 in range(0, T, tc_len)]

    with tile.TileContext(nc) as tctx:
        with (
            tctx.tile_pool(name="par", bufs=2) as par_pool,
            tctx.tile_pool(name="blk", bufs=2) as blk_pool,
            tctx.tile_pool(name="st", bufs=4) as st_pool,
            tctx.tile_pool(name="per", bufs=1) as per_pool,
        ):
            V = nc.vector
            G = nc.gpsimd
            A = nc.scalar
            S = nc.sync

            # Seed the ACT table containing BOTH Ln and Exp (and Relu/Copy):
            # natural_log_exp_and_others, set id 6. Without this the
            # table-load pass ping-pongs ln-only/exp-only tables per step.
            A.add_instruction(
                mybir.InstLoadActFuncSet(
                    name=nc.get_next_instruction_name(),
                    act_func_set_id=6, ins=[], outs=[],
                )
            )

            def tt(eng, out, a, b, o):
                eng.tensor_tensor(out, a, b, o)

            Qfull = per_pool.tile([PPART, (LENF - 1 + T) * CL], F32)
            uh_t = per_pool.tile([PPART, LENF * CL], F32)
            ones = per_pool.tile([PPART, 16], F32)
            S.dma_start(uh_t[:], uh[:])
            G.memset(Qfull[:, : (LENF - 1) * CL], 0.0)
            G.memset(ones[:], 1.0)

            state = {}
            # Snow states stored shifted by -NZ (so clamps become Relu).
            for s_, v0 in (("SPm", 0.001 - NZ), ("MWm", 0.001 - NZ),
                           ("SM", 0.001), ("SUZ", 0.001), ("SLZ", 0.001)):
                t_ = st_pool.tile([PPART, 16], F32, tag=s_)
                G.memset(t_[:], v0)
                state[s_] = t_

            def nt(tag, w=16):
                return st_pool.tile([PPART, w], F32, tag=tag, name=tag)

            def emit_tail(p):
                """Deferred response tail for step p['t']: runs on Pool/ACT at
                the start of step t+1 so Pool never stalls on the DVE cycle.
                Returns the Q048 tile for the deferred DVE reduce."""
                if p is None:
                    return None
                qp_ = nt("qp")
                tt(G, qp_[:], p["SUZ2"][:], p["UZL"], op.subtract)
                qm_ = nt("qm")
                A.activation(qm_[:], qp_[:], AF.Relu)
                Q048 = nt("Q048", 48)
                tt(G, Q048[:, 0:16], qm_[:], p["K0"], op.mult)          # Q0
                QS = p["QS"]          # [SUZ3 | SLZ2]; SLZ2 already written
                tt(G, QS[:, 0:16], p["SUZ2"][:], Q048[:, 0:16], op.subtract)
                SUZn = nt("SUZ")
                tt(G, SUZn[:], p["K1c"], QS[:, 0:16], op.mult)
                state["SUZ"] = SUZn
                tt(G, Q048[:, 16:48], p["kk2"], QS[:], op.mult)  # Q1|Q2
                p["Q048"] = Q048
                return p

            pend = None

            for (t0, tcn) in chunks:
                n16 = tcn * 16
                # ---- chunk DMAs ----
                part = {}
                for k in range(12):
                    pt = par_pool.tile([PPART, tc_len * 16], F32, tag=f"par{k}",
                                       name=f"par{k}_{t0}")
                    S.dma_start(
                        pt[:, :n16].rearrange("p (t c m) -> p t c m", c=CL, m=M),
                        pp[k, :, t0 : t0 + tcn, :, :],
                    )
                    part[k] = pt
                kkt = par_pool.tile([PPART, tc_len * 32], F32, tag="kk",
                                    name=f"kk_{t0}")
                S.dma_start(
                    kkt[:, : tcn * 32].rearrange(
                        "p (t k c m) -> p t k c m", k=2, c=CL, m=M),
                    kk[:, t0 : t0 + tcn, :, :, :],
                )
                xft = {}
                for c in range(3):
                    xt = blk_pool.tile([PPART, tc_len * CL], F32, tag=f"xf{c}",
                                       name=f"xf{c}_{t0}")
                    S.dma_start(
                        xt[:, : tcn * CL].rearrange("p (t c) -> p t c", c=CL),
                        xf[c, :, t0 : t0 + tcn, :],
                    )
                    xft[c] = xt

                # ---- parameter scaling in-place (ACT) ----
                for k, (_, sc_, bi_) in enumerate(PP_ROWS):
                    if sc_ == 1.0 and bi_ == 0.0:
                        continue
                    A.activation(part[k][:, :n16], part[k][:, :n16], AF.Copy,
                                 bias=float(bi_), scale=float(sc_))

                def bc4(xtile):
                    return (
                        xtile[:, : tcn * CL]
                        .rearrange("p (t c) -> p t c", c=CL)
                        .unsqueeze(3)
                        .to_broadcast((PPART, tcn, CL, M))
                    )

                def f4(btile):
                    return btile[:, :n16].rearrange(
                        "p (t c m) -> p t c m", c=CL, m=M
                    )

                Pb = bc4(xft[0])
                TAb = bc4(xft[1])
                PETb = bc4(xft[2])

                def bt(tag):
                    return blk_pool.tile([PPART, tc_len * 16], F32, tag=tag,
                                         name=f"{tag}_{t0}")

                # ---- bulk derived ----
                Gt = bt("Gt")
                tt(G, f4(Gt), TAb, f4(part[I_TT]), op.subtract)     # Ta - TT
                maskt = bt("mask")
                tt(V, f4(maskt), TAb, f4(part[I_TT]), op.is_ge)
                RAIN = bt("RAIN")
                tt(G, f4(RAIN), f4(maskt), Pb, op.mult)
                SNOW = bt("SNOW")
                tt(G, f4(SNOW), Pb, f4(RAIN), op.subtract)
                gc0 = bt("gc0")
                tt(G, gc0[:, :n16], part[I_CFMAX][:, :n16], Gt[:, :n16], op.mult)
                Gc = bt("Gc")
                A.activation(Gc[:, :n16], gc0[:, :n16], AF.Relu)
                CFMXn = bt("CFMXn")
                tt(G, CFMXn[:, :n16], part[I_CFRN][:, :n16],
                   part[I_CFMAX][:, :n16], op.mult)
                rc0 = bt("rc0")
                tt(G, rc0[:, :n16], CFMXn[:, :n16], Gt[:, :n16], op.mult)
                Rc = bt("Rc")
                A.activation(Rc[:, :n16], rc0[:, :n16], AF.Relu)

                FCinv = bt("FCinv")
                scr = bt("scr")
                V.reciprocal_approx_accurate(FCinv[:, :n16],
                                             part[I_FC][:, :n16],
                                             scr[:, :n16])
                CFCinv = bt("CFCinv")
                tt(V, CFCinv[:, :n16], part[I_C][:, :n16], FCinv[:, :n16],
                   op.mult)
                Cc = bt("Cc")
                V.tensor_scalar(Cc[:, :n16], part[I_C][:, :n16], -1.0, 1.0,
                                op0=op.mult, op1=op.add)        # 1 - C
                K1c = bt("K1c")
                kk1 = kkt[:, : tcn * 32].rearrange(
                    "p (t k x) -> p t k x", k=2, x=16)
                V.tensor_scalar(
                    K1c[:, :n16].rearrange("p (t x) -> p t x", x=16),
                    kk1[:, :, 0, :], -1.0, 1.0, op0=op.mult, op1=op.add)
                K2c = bt("K2c")
                V.tensor_scalar(
                    K2c[:, :n16].rearrange("p (t x) -> p t x", x=16),
                    kk1[:, :, 1, :], -1.0, 1.0, op0=op.mult, op1=op.add)
                lnFC = bt("lnFC")
                A.activation(lnFC[:, :n16], part[I_FC][:, :n16], AF.Ln)
                BlnFC = bt("BlnFC")
                tt(G, BlnFC[:, :n16], part[I_BETA][:, :n16], lnFC[:, :n16],
                   op.mult)
                LPFC = bt("LPFC")
                tt(G, LPFC[:, :n16], part[I_LP][:, :n16], part[I_FC][:, :n16],
                   op.mult)
                lnLPFC = bt("lnLPFC")
                A.activation(lnLPFC[:, :n16], LPFC[:, :n16], AF.Ln)
                BlnLPFC = bt("BlnLPFC")
                tt(G, BlnLPFC[:, :n16], part[I_BETAET][:, :n16],
                   lnLPFC[:, :n16], op.mult)

                # ---- sequential steps ----
                for ti in range(tcn):
                    t = t0 + ti
                    sl = slice(ti * 16, (ti + 1) * 16)
                    sl2 = slice(ti * 32, (ti + 1) * 32)

                    def ps(k):
                        return part[k][:, sl]

                    SPm, MWm = state["SPm"], state["MWm"]
                    SM, SLZ = state["SM"], state["SLZ"]

                    # -- snow scan (Pool TT + ACT Relu; states shifted -NZ) --
                    SP1m = nt("SP1m")
                    tt(G, SP1m[:], SPm[:], SNOW[:, sl], op.add)
                    dd1 = nt("dd1")
                    tt(G, dd1[:], SP1m[:], Gc[:, sl], op.subtract)
                    m1 = nt("m1")
                    A.activation(m1[:], dd1[:], AF.Relu)      # max(d1,NZ)-NZ
                    melt = nt("melt")
                    tt(G, melt[:], SP1m[:], m1[:], op.subtract)
                    MW1m = nt("MW1m")
                    tt(G, MW1m[:], MWm[:], melt[:], op.add)
                    dd2 = nt("dd2")
                    tt(G, dd2[:], MW1m[:], Rc[:, sl], op.subtract)
                    m2 = nt("m2")
                    A.activation(m2[:], dd2[:], AF.Relu)      # MW2 - NZ
                    rfz = nt("rfz")
                    tt(G, rfz[:], MW1m[:], m2[:], op.subtract)
                    SP3m = nt("SPm")
                    tt(G, SP3m[:], m1[:], rfz[:], op.add)
                    state["SPm"] = SP3m
                    W = nt("W")
                    tt(G, W[:], ps(I_CWH), SP3m[:], op.mult)
                    dd3 = nt("dd3")
                    tt(G, dd3[:], m2[:], W[:], op.subtract)
                    tos = nt("tos")
                    A.activation(tos[:], dd3[:], AF.Relu)
                    MW3m = nt("MWm")
                    tt(G, MW3m[:], m2[:], tos[:], op.subtract)
                    state["MWm"] = MW3m
                    wi = nt("wi")
                    tt(G, wi[:], RAIN[:, sl], tos[:], op.add)

                    # -- deferred response tail of step t-1 (Pool + ACT) --
                    done = emit_tail(pend)

                    # -- helpers (Pool; SUS needs SUZ updated by the tail) --
                    SMa = nt("SMa")
                    tt(G, SMa[:], SM[:], wi[:], op.add)
                    SUS = nt("SUS")
                    tt(G, SUS[:], state["SUZ"][:], SMa[:], op.add)

                    # -- SLZ-linearized capillary factors (DVE, off-cycle) --
                    t1 = nt("t1")
                    tt(V, t1[:], CFCinv[:, sl], SLZ[:], op.mult)  # C*SLZ/FC
                    t2 = nt("t2")
                    tt(V, t2[:], ones[:], t1[:], op.subtract)     # 1-C*SLZ/FC
                    t3 = nt("t3")
                    tt(V, t3[:], ps(I_C), SLZ[:], op.mult)        # C*SLZ
                    z1 = nt("z1")
                    tt(V, z1[:], Cc[:, sl], SLZ[:], op.mult)      # (1-C)*SLZ

                    # -- soil critical cycle (DVE + ACT) --
                    lna = nt("lna")
                    A.activation(lna[:], SM[:], AF.Ln)
                    u1 = nt("u1")
                    tt(V, u1[:], ps(I_BETA), lna[:], op.mult)
                    e1 = nt("e1")
                    tt(V, e1[:], u1[:], BlnFC[:, sl], op.subtract)
                    E1 = nt("E1")
                    A.activation(E1[:], e1[:], AF.Exp)           # (SM/FC)^B
                    rech = nt("rech")
                    V.scalar_tensor_tensor(rech[:], E1[:], 1.0, wi[:],
                                           op.min, op.mult)
                    SM1 = nt("SM1")
                    tt(V, SM1[:], SMa[:], rech[:], op.subtract)
                    lnb = nt("lnb")
                    A.activation(lnb[:], SM1[:], AF.Ln)
                    u2 = nt("u2")
                    tt(V, u2[:], ps(I_BETAET), lnb[:], op.mult)
                    e2 = nt("e2")
                    tt(V, e2[:], u2[:], BlnLPFC[:, sl], op.subtract)
                    E2 = nt("E2")
                    A.activation(E2[:], e2[:], AF.Exp)
                    SMc = nt("SMc")
                    tt(V, SMc[:], SM1[:], ps(I_FC), op.min)
                    pe = nt("pe")
                    V.scalar_tensor_tensor(
                        pe[:].rearrange("p (c m) -> p c m", m=M),
                        E2[:].rearrange("p (c m) -> p c m", m=M), 1.0,
                        PETb[:, ti, :, :], op.min, op.mult)
                    SM3p = nt("SM3p")
                    tt(V, SM3p[:], SMc[:], pe[:], op.subtract)
                    # SM4 = max(SM3',NZ)*t2 + t3; SLZ1 = z1 + max(SM3',NZ)*t1
                    SM4a = nt("SM4a")
                    V.scalar_tensor_tensor(SM4a[:], SM3p[:], NZ, t2[:],
                                           op.max, op.mult)
                    SM4 = nt("SM")
                    tt(V, SM4[:], SM4a[:], t3[:], op.add)
                    state["SM"] = SM4
                    sl1 = nt("sl1")
                    V.scalar_tensor_tensor(sl1[:], SM3p[:], NZ, t1[:],
                                           op.max, op.mult)
                    SLZ1 = nt("SLZ1")
                    tt(V, SLZ1[:], sl1[:], z1[:], op.add)

                    # -- response head (DVE) --
                    SUZ1 = nt("SUZ1")
                    tt(V, SUZ1[:], SUS[:], SMc[:], op.subtract)
                    PERCa = nt("PERCa")
                    tt(V, PERCa[:], SUZ1[:], ps(I_PERC), op.min)
                    QS = nt("QS", 32)     # [SUZ3 | SLZ2]; SUZ3 set in tail
                    V.scalar_tensor_tensor(QS[:, 16:32], SLZ1[:], NZ,
                                           PERCa[:], op.max, op.add)  # SLZ2
                    SLZn = nt("SLZ")
                    tt(V, SLZn[:], K2c[:, sl], QS[:, 16:32], op.mult)
                    state["SLZ"] = SLZn
                    SUZ2 = nt("SUZ2")
                    tt(V, SUZ2[:], SUZ1[:], PERCa[:], op.subtract)

                    # -- deferred reduce of step t-1 (DVE) --
                    if done is not None:
                        V.tensor_reduce(
                            Qfull[:, (LENF - 1 + done["t"]) * CL
                                  : (LENF + done["t"]) * CL],
                            done["Q048"][:].rearrange(
                                "p (b c m) -> p c b m", b=3, c=CL, m=M),
                            axis=mybir.AxisListType.XY,
                            op=op.add,
                        )

                    pend = {
                        "t": t, "SUZ2": SUZ2, "QS": QS,
                        "UZL": ps(I_UZL), "K0": ps(I_K0),
                        "K1c": K1c[:, sl], "kk2": kkt[:, sl2],
                    }

            # ---- final deferred tail + reduce ----
            done = emit_tail(pend)
            if done is not None:
                V.tensor_reduce(
                    Qfull[:, (LENF - 1 + done["t"]) * CL
                          : (LENF + done["t"]) * CL],
                    done["Q048"][:].rearrange(
                        "p (b c m) -> p c b m", b=3, c=CL, m=M),
                    axis=mybir.AxisListType.XY,
                    op=op.add,
                )

            # ---- gamma-UH routing (DVE, bulk) ----
            Qr = per_pool.tile([PPART, T * CL], F32)
            prod = per_pool.tile([PPART, T * CL], F32)

            def qr4(ap_):
                return ap_.rearrange("p (t c) -> p t c", c=CL)

            for k in range(LENF):
                sh = Qfull[:, (LENF - 1 - k) * CL : (LENF - 1 - k + T) * CL]
                uhk = (
                    uh_t[:, k * CL : (k + 1) * CL]
                    .unsqueeze(1)
                    .to_broadcast((PPART, T, CL))
                )
                if k == 0:
                    tt(V, qr4(Qr[:]), uhk, qr4(sh), op.mult)
                else:
                    tt(V, qr4(prod[:]), uhk, qr4(sh), op.mult)
                    tt(V, qr4(Qr[:]), qr4(Qr[:]), qr4(prod[:]), op.add)

            S.dma_start(qr[:, :, :], Qr[:].rearrange("p (t c) -> p t c", c=CL))

    return nc


# ---------------- host-side packing ----------------

def pack_inputs(x_hydro_model, params_raw, conv_params_hydro):
    T = x_hydro_model.shape[0]
    f32 = np.float32
    x = np.ascontiguousarray(x_hydro_model, dtype=f32)
    xs = x.reshape(T, NCORES, PPART, CL, 3).transpose(1, 4, 2, 0, 3)

    idx = [r[0] for r in PP_ROWS]
    pr = np.ascontiguousarray(params_raw[:, :, idx, :], dtype=f32)
    prs = pr.reshape(T, NCORES, PPART, CL, 12, M).transpose(1, 4, 2, 0, 3, 5)

    # K1/K2 pre-scaled, packed [PPART, T, 2, CL, M]
    k12 = np.ascontiguousarray(params_raw[:, :, 3:5, :], dtype=np.float64)
    k12 = k12 * np.array([0.49, 0.199]).reshape(1, 1, 2, 1) + np.array(
        [0.01, 0.001]).reshape(1, 1, 2, 1)
    k12 = k12.astype(f32).reshape(T, NCORES, PPART, CL, 2, M).transpose(
        1, 2, 0, 4, 3, 5)  # [core, PPART, T, 2, CL, M]

    conv = np.asarray(conv_params_hydro, dtype=np.float64)
    a = conv[:, 0] * 2.9
    b = conv[:, 1] * 6.5
    aa = np.maximum(a, 0) + 0.1
    theta = np.maximum(b, 0) + 0.5
    tgrid = np.arange(0.5, float(LENF), dtype=np.float64)[:, None]
    lg = np.array([math.lgamma(v) for v in aa])
    w = np.exp(-lg) / theta ** aa * tgrid ** (aa - 1.0) * np.exp(-tgrid / theta)
    w = w / w.sum(0)
    UH = (w * (1.0 / M)).astype(f32)  # [LENF, NGRID], mean-over-M folded in
    uh_c = UH.reshape(LENF, NCORES, PPART, CL).transpose(1, 2, 0, 3)

    in_maps = []
    for i in range(NCORES):
        in_maps.append({
            "pp": np.ascontiguousarray(prs[i]),
            "kk": np.ascontiguousarray(k12[i]),
            "xf": np.ascontiguousarray(xs[i]),
            "uh": np.ascontiguousarray(uh_c[i]).reshape(PPART, LENF * CL),
        })
    return in_maps


def unpack_outputs(results, T):
    out = np.empty((T, NGRID), np.float32)
    for i in range(NCORES):
        q = results[i]["qr"].reshape(PPART, T, CL)
        out[:, i * NSH : (i + 1) * NSH] = q.transpose(1, 0, 2).reshape(T, NSH)
    return out


_PROG_CACHE = {}


def kernel(x_hydro_model, params_raw, conv_params_hydro):
    from concourse.bass_utils import run_bass_kernel_spmd

    T = x_hydro_model.shape[0]
    key = T
    if key not in _PROG_CACHE:
        _PROG_CACHE[key] = build_program(T=T)
    nc = _PROG_CACHE[key]
    if not nc.is_finalized():
        nc.finalize()
    in_maps = pack_inputs(x_hydro_model, params_raw, conv_params_hydro)
    res = run_bass_kernel_spmd(nc, in_maps, list(range(NCORES)))
    return unpack_outputs(res.results, T)


# revision 8
# speedup vs baseline: 1.7134x; 1.0471x over previous
"""HBV hydrological model (nn_HBVMulTDET_WaterLoss) as a Bass/Tile kernel on
8 Trainium2 NeuronCores.

Strategy: pure data parallelism over the 4000 grid cells (500 cells/core).
Per-core layout: partition p in [0,125) holds 4 cells x 4 components = 16
state lanes in the free dim. The T=365 recurrence is a fully unrolled
instruction stream balanced across three engines:
  - DVE: the soil-moisture critical cycle (pow via ln/exp affine, recharge,
    evap, capillary) with NZ-clamps fused into scalar_tensor_tensor ops.
  - Pool (GPSIMD): snow pack/meltwater scan + upper/lower-zone response,
    using only tensor_tensor add/sub/mult (the only TT ops Pool supports).
  - ACT: all max(x,0)-style clamps as Relu, plus Ln/Exp for the two powers.
    A single activation table (natural_log_exp_and_others, set id 6) is
    seeded explicitly so the compiler never reloads tables (1.28us each).
Algebraic restructurings vs the straight reference:
  - recharge/excess are never materialized: SUZ1 = (SUZ + SM + wi) - SMc.
  - capillary's min(.,1) and relu are provably no-ops (C<=1, SM3<=FC);
    SM4 and SLZ1 are computed from SLZ-linearized forms
    SM4 = max(SM3',NZ)*(1 - C*SLZ/FC) + C*SLZ, SLZ1 = SLZ*(1-C) + s*C*SLZ/FC
    whose SLZ-dependent factors are computed off the critical path.
  - evap factor uses SM1 instead of min(SM1,FC): identical after clip-to-1.
  - snow states are stored shifted by -NEARZERO so every clamp is a Relu.
  - x^b = exp(b*ln(x) - b*ln(FC)) with b*ln(FC) hoisted to bulk precompute.
Gamma unit-hydrograph weights are computed on host (tiny [15,4000]
preprocessing of conv_params); the routing convolution runs on device.
"""
import math
import numpy as np

T_FULL = 365
NGRID = 4000
NCORES = 8
NSH = NGRID // NCORES      # 500 cells per core
PPART = 125                # partitions used
CL = 4                     # cells per partition
M = 4                      # nmul components
LENF = 15
NZ = 1e-5
TC = 32                    # time-chunk length

# pp rows (param index in params_raw, scale, bias); K1/K2 ship separately
# pre-scaled as the packed "kk" tensor. CFR is sign-folded (negated).
PP_ROWS = [
    (0, 5.0, 1.0),       # 0 BETA
    (1, 950.0, 50.0),    # 1 FC
    (2, 0.85, 0.05),     # 2 K0
    (5, 0.8, 0.2),       # 3 LP
    (6, 10.0, 0.0),      # 4 PERC
    (7, 100.0, 0.0),     # 5 UZL
    (8, 5.0, -2.5),      # 6 TT
    (9, 9.5, 0.5),       # 7 CFMAX
    (10, -0.1, 0.0),     # 8 CFRn = -CFR
    (11, 0.2, 0.0),      # 9 CWH
    (12, 4.7, 0.3),      # 10 BETAET
    (13, 1.0, 0.0),      # 11 C
]
I_BETA, I_FC, I_K0, I_LP, I_PERC, I_UZL, I_TT, I_CFMAX, I_CFRN, I_CWH, \
    I_BETAET, I_C = range(12)


def build_program(T=T_FULL, tc_len=TC):
    import concourse.bass as bass
    import concourse.bacc as bacc
    import concourse.mybir as mybir
    import concourse.tile as tile

    F32 = mybir.dt.float32
    op = mybir.AluOpType
    AF = mybir.ActivationFunctionType

    nc = bacc.Bacc("TRN2")
    pp = nc.declare_dram_parameter("pp", [12, PPART, T, CL, M], F32, isOutput=False)
    kk = nc.declare_dram_parameter("kk", [PPART, T, 2, CL, M], F32, isOutput=False)
    xf = nc.declare_dram_parameter("xf", [3, PPART, T, CL], F32, isOutput=False)
    uh = nc.declare_dram_parameter("uh", [PPART, LENF * CL], F32, isOutput=False)
    qr = nc.declare_dram_parameter("qr", [PPART, T, CL], F32, isOutput=True)

    chunks = [(t0, min(tc_len, T - t0)) for t0 in range(0, T, tc_len)]

    with tile.TileContext(nc) as tctx:
        with (
            tctx.tile_pool(name="par", bufs=2) as par_pool,
            tctx.tile_pool(name="blk", bufs=2) as blk_pool,
            tctx.tile_pool(name="st", bufs=4) as st_pool,
            tctx.tile_pool(name="per", bufs=1) as per_pool,
        ):
            V = nc.vector
            G = nc.gpsimd
            A = nc.scalar
            S = nc.sync

            # Seed the ACT table containing BOTH Ln and Exp (and Relu/Copy):
            # natural_log_exp_and_others, set id 6. Without this the
            # table-load pass ping-pongs ln-only/exp-only tables per step.
            A.add_instruction(
                mybir.InstLoadActFuncSet(
                    name=nc.get_next_instruction_name(),
                    act_func_set_id=6, ins=[], outs=[],
                )
            )

            def tt(eng, out, a, b, o):
                eng.tensor_tensor(out, a, b, o)

            Qfull = per_pool.tile([PPART, (LENF - 1 + T) * CL], F32)
            uh_t = per_pool.tile([PPART, LENF * CL], F32)
            ones = per_pool.tile([PPART, 16], F32)
            S.dma_start(uh_t[:], uh[:])
            G.memset(Qfull[:, : (LENF - 1) * CL], 0.0)
            G.memset(ones[:], 1.0)

            state = {}
            # Snow states stored shifted by -NZ (so clamps become Relu).
            for s_, v0 in (("SPm", 0.001 - NZ), ("MWm", 0.001 - NZ),
                           ("SM", 0.001), ("SUZ", 0.001), ("SLZ", 0.001)):
                t_ = st_pool.tile([PPART, 16], F32, tag=s_)
                G.memset(t_[:], v0)
                state[s_] = t_

            def nt(tag, w=16):
                return st_pool.tile([PPART, w], F32, tag=tag, name=tag)

            pend = None     # deferred response tail of step t-1
            snow_out = {}   # t -> wi tile (snow runs one step ahead)

            for (t0, tcn) in chunks:
                n16 = tcn * 16
                # ---- chunk DMAs ----
                part = {}
                for k in range(12):
                    pt = par_pool.tile([PPART, tc_len * 16], F32, tag=f"par{k}",
                                       name=f"par{k}_{t0}")
                    S.dma_start(
                        pt[:, :n16].rearrange("p (t c m) -> p t c m", c=CL, m=M),
                        pp[k, :, t0 : t0 + tcn, :, :],
                    )
                    part[k] = pt
                kkt = par_pool.tile([PPART, tc_len * 32], F32, tag="kk",
                                    name=f"kk_{t0}")
                S.dma_start(
                    kkt[:, : tcn * 32].rearrange(
                        "p (t k c m) -> p t k c m", k=2, c=CL, m=M),
                    kk[:, t0 : t0 + tcn, :, :, :],
                )
                xft = {}
                for c in range(3):
                    xt = blk_pool.tile([PPART, tc_len * CL], F32, tag=f"xf{c}",
                                       name=f"xf{c}_{t0}")
                    S.dma_start(
                        xt[:, : tcn * CL].rearrange("p (t c) -> p t c", c=CL),
                        xf[c, :, t0 : t0 + tcn, :],
                    )
                    xft[c] = xt

                # ---- parameter scaling in-place (ACT) ----
                for k, (_, sc_, bi_) in enumerate(PP_ROWS):
                    if sc_ == 1.0 and bi_ == 0.0:
                        continue
                    A.activation(part[k][:, :n16], part[k][:, :n16], AF.Copy,
                                 bias=float(bi_), scale=float(sc_))

                def bc4(xtile):
                    return (
                        xtile[:, : tcn * CL]
                        .rearrange("p (t c) -> p t c", c=CL)
                        .unsqueeze(3)
                        .to_broadcast((PPART, tcn, CL, M))
                    )

                def f4(btile):
                    return btile[:, :n16].rearrange(
                        "p (t c m) -> p t c m", c=CL, m=M
                    )

                Pb = bc4(xft[0])
                TAb = bc4(xft[1])
                PETb = bc4(xft[2])

                def bt(tag):
                    return blk_pool.tile([PPART, tc_len * 16], F32, tag=tag,
                                         name=f"{tag}_{t0}")

                # ---- bulk derived ----
                Gt = bt("Gt")
                tt(G, f4(Gt), TAb, f4(part[I_TT]), op.subtract)     # Ta - TT
                maskt = bt("mask")
                tt(V, f4(maskt), TAb, f4(part[I_TT]), op.is_ge)
                RAIN = bt("RAIN")
                tt(G, f4(RAIN), f4(maskt), Pb, op.mult)
                SNOW = bt("SNOW")
                tt(G, f4(SNOW), Pb, f4(RAIN), op.subtract)
                gc0 = bt("gc0")
                tt(G, gc0[:, :n16], part[I_CFMAX][:, :n16], Gt[:, :n16], op.mult)
                Gc = bt("Gc")
                A.activation(Gc[:, :n16], gc0[:, :n16], AF.Relu)
                CFMXn = bt("CFMXn")
                tt(G, CFMXn[:, :n16], part[I_CFRN][:, :n16],
                   part[I_CFMAX][:, :n16], op.mult)
                rc0 = bt("rc0")
                tt(G, rc0[:, :n16], CFMXn[:, :n16], Gt[:, :n16], op.mult)
                Rc = bt("Rc")
                A.activation(Rc[:, :n16], rc0[:, :n16], AF.Relu)

                FCinv = bt("FCinv")
                scr = bt("scr")
                V.reciprocal_approx_accurate(FCinv[:, :n16],
                                             part[I_FC][:, :n16],
                                             scr[:, :n16])
                CFCinv = bt("CFCinv")
                tt(V, CFCinv[:, :n16], part[I_C][:, :n16], FCinv[:, :n16],
                   op.mult)
                Cc = bt("Cc")
                V.tensor_scalar(Cc[:, :n16], part[I_C][:, :n16], -1.0, 1.0,
                                op0=op.mult, op1=op.add)        # 1 - C
                K1c = bt("K1c")
                kk1 = kkt[:, : tcn * 32].rearrange(
                    "p (t k x) -> p t k x", k=2, x=16)
                V.tensor_scalar(
                    K1c[:, :n16].rearrange("p (t x) -> p t x", x=16),
                    kk1[:, :, 0, :], -1.0, 1.0, op0=op.mult, op1=op.add)
                K2c = bt("K2c")
                V.tensor_scalar(
                    K2c[:, :n16].rearrange("p (t x) -> p t x", x=16),
                    kk1[:, :, 1, :], -1.0, 1.0, op0=op.mult, op1=op.add)
                lnFC = bt("lnFC")
                A.activation(lnFC[:, :n16], part[I_FC][:, :n16], AF.Ln)
                BlnFC = bt("BlnFC")
                tt(G, BlnFC[:, :n16], part[I_BETA][:, :n16], lnFC[:, :n16],
                   op.mult)
                LPFC = bt("LPFC")
                tt(G, LPFC[:, :n16], part[I_LP][:, :n16], part[I_FC][:, :n16],
                   op.mult)
                lnLPFC = bt("lnLPFC")
                A.activation(lnLPFC[:, :n16], LPFC[:, :n16], AF.Ln)
                BlnLPFC = bt("BlnLPFC")
                tt(G, BlnLPFC[:, :n16], part[I_BETAET][:, :n16],
                   lnLPFC[:, :n16], op.mult)

                def emit_snow(ti_):
                    """Snow scan for step t0+ti_ (Pool TT + ACT Relu),
                    pipelined one step ahead of the soil cycle. States are
                    shifted by -NZ so every clamp is a Relu."""
                    sl_ = slice(ti_ * 16, (ti_ + 1) * 16)
                    SPm, MWm = state["SPm"], state["MWm"]
                    SP1m = nt("SP1m")
                    tt(G, SP1m[:], SPm[:], SNOW[:, sl_], op.add)
                    dd1 = nt("dd1")
                    tt(G, dd1[:], SP1m[:], Gc[:, sl_], op.subtract)
                    m1 = nt("m1")
                    A.activation(m1[:], dd1[:], AF.Relu)    # max(d1,NZ)-NZ
                    melt = nt("melt")
                    tt(G, melt[:], SP1m[:], m1[:], op.subtract)
                    MW1m = nt("MW1m")
                    tt(G, MW1m[:], MWm[:], melt[:], op.add)
                    dd2 = nt("dd2")
                    tt(G, dd2[:], MW1m[:], Rc[:, sl_], op.subtract)
                    m2 = nt("m2")
                    A.activation(m2[:], dd2[:], AF.Relu)    # MW2 - NZ
                    rfz = nt("rfz")
                    tt(G, rfz[:], MW1m[:], m2[:], op.subtract)
                    SP3m = nt("SPm")
                    tt(G, SP3m[:], m1[:], rfz[:], op.add)
                    state["SPm"] = SP3m
                    W = nt("W")
                    tt(G, W[:], part[I_CWH][:, sl_], SP3m[:], op.mult)
                    dd3 = nt("dd3")
                    tt(G, dd3[:], m2[:], W[:], op.subtract)
                    tos = nt("tos")
                    A.activation(tos[:], dd3[:], AF.Relu)
                    MW3m = nt("MWm")
                    tt(G, MW3m[:], m2[:], tos[:], op.subtract)
                    state["MWm"] = MW3m
                    wi = nt("wi")
                    tt(G, wi[:], RAIN[:, sl_], tos[:], op.add)
                    snow_out[t0 + ti_] = wi

                # ---- sequential steps ----
                for ti in range(tcn):
                    t = t0 + ti
                    sl = slice(ti * 16, (ti + 1) * 16)
                    sl2 = slice(ti * 32, (ti + 1) * 32)

                    def ps(k):
                        return part[k][:, sl]

                    if ti == 0:
                        emit_snow(0)    # snow not yet pipelined across chunks

                    SM, SLZ = state["SM"], state["SLZ"]
                    wi = snow_out.pop(t)
                    p = pend

                    # -- ACT: cycle ln/exp; qm + next-step snow relus fill
                    #    the wait windows (emitted inside emit_snow). --
                    lna = nt("lna")
                    A.activation(lna[:], SM[:], AF.Ln)

                    # -- Pool: deferred response tail of t-1, then helpers --
                    if p is not None:
                        qp_ = nt("qp")
                        tt(G, qp_[:], p["SUZ2"][:], p["UZL"], op.subtract)
                        qm_ = nt("qm")
                        A.activation(qm_[:], qp_[:], AF.Relu)
                        Q048 = nt("Q048", 48)
                        tt(G, Q048[:, 0:16], qm_[:], p["K0"], op.mult)  # Q0
                        QS_ = p["QS"]   # [SUZ3 | SLZ2]; SLZ2 already there
                        tt(G, QS_[:, 0:16], p["SUZ2"][:], Q048[:, 0:16],
                           op.subtract)
                        SUZn = nt("SUZ")
                        tt(G, SUZn[:], p["K1c"], QS_[:, 0:16], op.mult)
                        state["SUZ"] = SUZn
                        tt(G, Q048[:, 16:48], p["kk2"], QS_[:], op.mult)
                        p["Q048"] = Q048
                    SMa = nt("SMa")
                    tt(G, SMa[:], SM[:], wi[:], op.add)
                    SUS = nt("SUS")
                    tt(G, SUS[:], state["SUZ"][:], SMa[:], op.add)

                    # -- DVE cycle; off-cycle ops placed into wait windows --
                    u1 = nt("u1")
                    tt(V, u1[:], ps(I_BETA), lna[:], op.mult)
                    e1 = nt("e1")
                    tt(V, e1[:], u1[:], BlnFC[:, sl], op.subtract)
                    E1 = nt("E1")
                    A.activation(E1[:], e1[:], AF.Exp)          # (SM/FC)^B
                    # window filler: capillary factors from SLZ(t-1)
                    t1 = nt("t1")
                    tt(V, t1[:], CFCinv[:, sl], SLZ[:], op.mult)
                    t2 = nt("t2")
                    tt(V, t2[:], ones[:], t1[:], op.subtract)
                    t3 = nt("t3")
                    tt(V, t3[:], ps(I_C), SLZ[:], op.mult)
                    z1 = nt("z1")
                    tt(V, z1[:], Cc[:, sl], SLZ[:], op.mult)
                    rech = nt("rech")
                    V.scalar_tensor_tensor(rech[:], E1[:], 1.0, wi[:],
                                           op.min, op.mult)
                    SM1 = nt("SM1")
                    tt(V, SM1[:], SMa[:], rech[:], op.subtract)
                    lnb = nt("lnb")
                    A.activation(lnb[:], SM1[:], AF.Ln)
                    # window filler: previous step's Q reduce
                    if p is not None:
                        V.tensor_reduce(
                            Qfull[:, (LENF - 1 + p["t"]) * CL
                                  : (LENF + p["t"]) * CL],
                            p["Q048"][:].rearrange(
                                "p (b c m) -> p c b m", b=3, c=CL, m=M),
                            axis=mybir.AxisListType.XY,
                            op=op.add,
                        )
                    u2 = nt("u2")
                    tt(V, u2[:], ps(I_BETAET), lnb[:], op.mult)
                    e2 = nt("e2")
                    tt(V, e2[:], u2[:], BlnLPFC[:, sl], op.subtract)
                    E2 = nt("E2")
                    A.activation(E2[:], e2[:], AF.Exp)
                    # window filler: SMc
                    SMc = nt("SMc")
                    tt(V, SMc[:], SM1[:], ps(I_FC), op.min)
                    # next-step snow emitted here: its Pool ops run after the
                    # tail/helpers above; its ACT relus queue after E2.
                    if ti + 1 < tcn:
                        emit_snow(ti + 1)
                    pe = nt("pe")
                    V.scalar_tensor_tensor(
                        pe[:].rearrange("p (c m) -> p c m", m=M),
                        E2[:].rearrange("p (c m) -> p c m", m=M), 1.0,
                        PETb[:, ti, :, :], op.min, op.mult)
                    SM3p = nt("SM3p")
                    tt(V, SM3p[:], SMc[:], pe[:], op.subtract)
                    # SM4 = max(SM3',NZ)*t2 + t3; SLZ1 = z1 + max(SM3',NZ)*t1
                    SM4a = nt("SM4a")
                    V.scalar_tensor_tensor(SM4a[:], SM3p[:], NZ, t2[:],
                                           op.max, op.mult)
                    SM4 = nt("SM")
                    tt(V, SM4[:], SM4a[:], t3[:], op.add)
                    state["SM"] = SM4
                    sl1 = nt("sl1")
                    V.scalar_tensor_tensor(sl1[:], SM3p[:], NZ, t1[:],
                                           op.max, op.mult)
                    SLZ1 = nt("SLZ1")
                    tt(V, SLZ1[:], sl1[:], z1[:], op.add)

                    # -- response head (DVE) --
                    SUZ1 = nt("SUZ1")
                    tt(V, SUZ1[:], SUS[:], SMc[:], op.subtract)
                    PERCa = nt("PERCa")
                    tt(V, PERCa[:], SUZ1[:], ps(I_PERC), op.min)
                    QS = nt("QS", 32)     # [SUZ3 | SLZ2]; SUZ3 set in tail
                    V.scalar_tensor_tensor(QS[:, 16:32], SLZ1[:], NZ,
                                           PERCa[:], op.max, op.add)  # SLZ2
                    SLZn = nt("SLZ")
                    tt(V, SLZn[:], K2c[:, sl], QS[:, 16:32], op.mult)
                    state["SLZ"] = SLZn
                    SUZ2 = nt("SUZ2")
                    tt(V, SUZ2[:], SUZ1[:], PERCa[:], op.subtract)

                    pend = {
                        "t": t, "SUZ2": SUZ2, "QS": QS,
                        "UZL": ps(I_UZL), "K0": ps(I_K0),
                        "K1c": K1c[:, sl], "kk2": kkt[:, sl2],
                    }

            # ---- final deferred tail + reduce ----
            p = pend
            if p is not None:
                qp_ = nt("qp")
                tt(G, qp_[:], p["SUZ2"][:], p["UZL"], op.subtract)
                qm_ = nt("qm")
                A.activation(qm_[:], qp_[:], AF.Relu)
                Q048 = nt("Q048", 48)
                tt(G, Q048[:, 0:16], qm_[:], p["K0"], op.mult)
                QS_ = p["QS"]
                tt(G, QS_[:, 0:16], p["SUZ2"][:], Q048[:, 0:16], op.subtract)
                tt(G, Q048[:, 16:48], p["kk2"], QS_[:], op.mult)
                V.tensor_reduce(
                    Qfull[:, (LENF - 1 + p["t"]) * CL : (LENF + p["t"]) * CL],
                    Q048[:].rearrange("p (b c m) -> p c b m", b=3, c=CL, m=M),
                    axis=mybir.AxisListType.XY,
                    op=op.add,
                )

            # ---- gamma-UH routing (DVE, bulk) ----
            Qr = per_pool.tile([PPART, T * CL], F32)
            prod = per_pool.tile([PPART, T * CL], F32)

            def qr4(ap_):
                return ap_.rearrange("p (t c) -> p t c", c=CL)

            for k in range(LENF):
                sh = Qfull[:, (LENF - 1 - k) * CL : (LENF - 1 - k + T) * CL]
                uhk = (
                    uh_t[:, k * CL : (k + 1) * CL]
                    .unsqueeze(1)
                    .to_broadcast((PPART, T, CL))
                )
                if k == 0:
                    tt(V, qr4(Qr[:]), uhk, qr4(sh), op.mult)
                else:
                    tt(V, qr4(prod[:]), uhk, qr4(sh), op.mult)
                    tt(V, qr4(Qr[:]), qr4(Qr[:]), qr4(prod[:]), op.add)

            S.dma_start(qr[:, :, :], Qr[:].rearrange("p (t c) -> p t c", c=CL))

    return nc


# ---------------- host-side packing ----------------

def pack_inputs(x_hydro_model, params_raw, conv_params_hydro):
    T = x_hydro_model.shape[0]
    f32 = np.float32
    x = np.ascontiguousarray(x_hydro_model, dtype=f32)
    xs = x.reshape(T, NCORES, PPART, CL, 3).transpose(1, 4, 2, 0, 3)

    idx = [r[0] for r in PP_ROWS]
    pr = np.ascontiguousarray(params_raw[:, :, idx, :], dtype=f32)
    prs = pr.reshape(T, NCORES, PPART, CL, 12, M).transpose(1, 4, 2, 0, 3, 5)

    # K1/K2 pre-scaled, packed [PPART, T, 2, CL, M]
    k12 = np.ascontiguousarray(params_raw[:, :, 3:5, :], dtype=np.float64)
    k12 = k12 * np.array([0.49, 0.199]).reshape(1, 1, 2, 1) + np.array(
        [0.01, 0.001]).reshape(1, 1, 2, 1)
    k12 = k12.astype(f32).reshape(T, NCORES, PPART, CL, 2, M).transpose(
        1, 2, 0, 4, 3, 5)  # [core, PPART, T, 2, CL, M]

    conv = np.asarray(conv_params_hydro, dtype=np.float64)
    a = conv[:, 0] * 2.9
    b = conv[:, 1] * 6.5
    aa = np.maximum(a, 0) + 0.1
    theta = np.maximum(b, 0) + 0.5
    tgrid = np.arange(0.5, float(LENF), dtype=np.float64)[:, None]
    lg = np.array([math.lgamma(v) for v in aa])
    w = np.exp(-lg) / theta ** aa * tgrid ** (aa - 1.0) * np.exp(-tgrid / theta)
    w = w / w.sum(0)
    UH = (w * (1.0 / M)).astype(f32)  # [LENF, NGRID], mean-over-M folded in
    uh_c = UH.reshape(LENF, NCORES, PPART, CL).transpose(1, 2, 0, 3)

    in_maps = []
    for i in range(NCORES):
        in_maps.append({
            "pp": np.ascontiguousarray(prs[i]),
            "kk": np.ascontiguousarray(k12[i]),
            "xf": np.ascontiguousarray(xs[i]),
            "uh": np.ascontiguousarray(uh_c[i]).reshape(PPART, LENF * CL),
        })
    return in_maps


def unpack_outputs(results, T):
    out = np.empty((T, NGRID), np.float32)
    for i in range(NCORES):
        q = results[i]["qr"].reshape(PPART, T, CL)
        out[:, i * NSH : (i + 1) * NSH] = q.transpose(1, 0, 2).reshape(T, NSH)
    return out


_PROG_CACHE = {}


def kernel(x_hydro_model, params_raw, conv_params_hydro):
    from concourse.bass_utils import run_bass_kernel_spmd

    T = x_hydro_model.shape[0]
    key = T
    if key not in _PROG_CACHE:
        _PROG_CACHE[key] = build_program(T=T)
    nc = _PROG_CACHE[key]
    if not nc.is_finalized():
        nc.finalize()
    in_maps = pack_inputs(x_hydro_model, params_raw, conv_params_hydro)
    res = run_bass_kernel_spmd(nc, in_maps, list(range(NCORES)))
    return unpack_outputs(res.results, T)
